# revision 2
# baseline (speedup 1.0000x reference)
"""LightGCN-style 3-layer sparse propagation on TRN2 (8 NeuronCores).

Row-sharded SpMM: each core owns a contiguous slab of output rows. Edges
are sorted by destination row, grouped into 64-row windows, and padded
into 128-edge tiles. A host-prebuilt values-folded one-hot indicator
[128 edges x 64 window-rows] (bf16, SBUF-resident) turns the per-window
segment-sum into a TensorE matmul accumulating in PSUM (one PSUM bank
per window pair, col-tiling for odd windows). Edge source rows are
gathered from a bf16 replica of x in DRAM via indirect DMA (one 128-row
tile per instruction). Between layers the new slabs are AllGathered.
Per-layer slabs are returned to the host, which sums layers, scales by
1/4, and does the final batch lookups (O(B*d), negligible).
"""

import math
import sys

import numpy as np

for _p in ("/root/.axon_site", "/root/.axon_site/_ro/trn_rl_repo",
           "/root/.axon_site/_ro/pypackages", "/opt/trn_rl_repo"):
    if _p not in sys.path:
        sys.path.append(_p)

import ml_dtypes

import concourse.bass as bass
import concourse.bacc as bacc
import concourse.tile as tile
from concourse import mybir
from concourse.bass_utils import run_bass_kernel_spmd

D = 64
WINDOW = 64
NCORES = 8
N_LAYERS = 3

NUM_USERS = 100000
NUM_ITEMS = 50000
N_NODES = NUM_USERS + NUM_ITEMS
N_PAD = 150528  # 8 * 18816; 18816 = 294*64 = 147*128

BF16 = mybir.dt.bfloat16
F32 = mybir.dt.float32
I32 = mybir.dt.int32


def _build_plan(rows, cols, values, n_pad, ck=64):
    slab = n_pad // NCORES
    nwin = slab // WINDOW
    blkn = slab // 128

    nnz = rows.shape[0]
    order = np.argsort(rows, kind="stable")
    r_s = rows[order].astype(np.int64)
    c_s = cols[order].astype(np.int64)
    v_s = values[order].astype(np.float32)

    gw = r_s // WINDOW
    group_sizes = np.bincount(gw, minlength=NCORES * nwin)
    counts = group_sizes.reshape(NCORES, nwin)
    T = np.maximum(1, np.ceil(counts.max(axis=0) / 128.0).astype(np.int64))
    G = int(T.sum())
    Gpad = ((G + ck - 1) // ck) * ck
    T = T.copy()
    T[-1] += Gpad - G
    first = np.concatenate([[0], np.cumsum(T)[:-1]]).astype(np.int64)
    last = (np.cumsum(T) - 1).astype(np.int64)
    win_of_tile = np.repeat(np.arange(nwin), T)

    group_start = np.concatenate([[0], np.cumsum(group_sizes)[:-1]])
    ordinal = np.arange(nnz) - group_start[gw]
    tile_within = ordinal // 128
    p = ordinal % 128
    core = gw // nwin
    local_w = gw % nwin
    tile_global = first[local_w] + tile_within
    local_row = r_s % WINDOW

    cv = c_s // slab
    rl = c_s % slab
    gidx = ((cv * 128 + rl % 128) * blkn + rl // 128).astype(np.int32)

    idx = np.zeros((NCORES, Gpad, 128), dtype=np.int32)
    ind = np.zeros((NCORES, Gpad, 128, WINDOW), dtype=ml_dtypes.bfloat16)
    idx[core, tile_global, p] = gidx
    ind[core, tile_global, p, local_row] = v_s
    ind = np.ascontiguousarray(ind.transpose(0, 2, 1, 3))

    return dict(
        slab=slab, nwin=nwin, blkn=blkn, Gpad=Gpad, ck=ck,
        T=T, first=first, last=last, win_of_tile=win_of_tile,
        idx=idx, ind=ind,
    )


def _wrap_slabs(x, n_pad):
    slab = n_pad // NCORES
    blkn = slab // 128
    return np.ascontiguousarray(
        x.reshape(NCORES, blkn, 128, D).transpose(0, 2, 1, 3)
        .reshape(NCORES, 128, blkn * D)
    )


def _unwrap_slab(xw, slab):
    blkn = slab // 128
    return xw.reshape(128, blkn, D).transpose(1, 0, 2).reshape(slab, D)


def _build_nc(plan, n_layers=N_LAYERS):
    nwin = plan["nwin"]
    blkn = plan["blkn"]
    Gpad = plan["Gpad"]
    ck = plan["ck"]
    first = plan["first"]
    last = plan["last"]
    win_of_tile = plan["win_of_tile"]
    slab_free = blkn * D
    chunks = Gpad // ck

    nc = bacc.Bacc("TRN2", target_bir_lowering=False, debug=False,
                   num_devices=NCORES)

    x0_d = nc.dram_tensor("x0slab", [128, slab_free], BF16,
                          kind="ExternalInput").ap()
    ind_d = nc.dram_tensor("ind", [128, Gpad * WINDOW], BF16,
                           kind="ExternalInput").ap()
    eidx_d = nc.dram_tensor("eidx", [chunks, 128, ck], I32,
                            kind="ExternalInput").ap()
    xouts = [
        nc.dram_tensor(f"xout{layer}", [128, slab_free], BF16,
                       kind="ExternalOutput").ap()
        for layer in range(n_layers)
    ]

    rg = [list(range(NCORES))]

    with tile.TileContext(nc) as tc:
        with tc.tile_pool(name="dram", bufs=1, space="DRAM") as dpool, \
             tc.tile_pool(name="const", bufs=1) as cpool, \
             tc.tile_pool(name="xnew", bufs=1) as xpool, \
             tc.tile_pool(name="gt", bufs=4) as gpool, \
             tc.tile_pool(name="idx", bufs=6) as ipool, \
             tc.tile_pool(name="ps", bufs=8, space="PSUM") as pspool:

            slab_bounce = dpool.tile([128, slab_free], BF16)
            xgs = [
                dpool.tile([128 * NCORES, slab_free], BF16,
                           addr_space="Shared", name=f"xg{layer}")
                for layer in range(n_layers)
            ]
            xvs = [xg.rearrange("a (b c) -> (a b) c", c=D) for xg in xgs]

            ind_sb = cpool.tile([128, Gpad * WINDOW], BF16)
            nc.sync.dma_start(out=ind_sb[:], in_=ind_d[:])
            nc.sync.dma_start(out=slab_bounce[:], in_=x0_d[:])
            nc.gpsimd.collective_compute(
                "AllGather", mybir.AluOpType.bypass, replica_groups=rg,
                ins=[slab_bounce.opt()], outs=[xgs[0].opt()],
            )

            for layer in range(n_layers):
                xnew = xpool.tile([128, slab_free], BF16)
                psum_tiles = {}
                for chi in range(chunks):
                    itile = ipool.tile([128, ck], I32)
                    nc.sync.dma_start(out=itile[:], in_=eidx_d[chi])
                    gtile = gpool.tile([128, ck * D], BF16)
                    for k in range(ck):
                        nc.gpsimd.indirect_dma_start(
                            out=gtile[:, k * D:(k + 1) * D],
                            out_offset=None,
                            in_=xvs[layer],
                            in_offset=bass.IndirectOffsetOnAxis(
                                ap=itile[:, k:k + 1], axis=0),
                        )
                    for k in range(ck):
                        t = chi * ck + k
                        w = int(win_of_tile[t])
                        pair = w // 2
                        half = w % 2
                        if pair not in psum_tiles:
                            psum_tiles[pair] = pspool.tile(
                                [128, 64], F32,
                                name=f"ps_l{layer}_p{pair}", tag="ps")
                        pt = psum_tiles[pair]
                        nc.tensor.matmul(
                            out=pt[64 * half:64 * half + 64, :],
                            lhsT=ind_sb[:, t * WINDOW:(t + 1) * WINDOW],
                            rhs=gtile[:, k * D:(k + 1) * D],
                            start=(t == first[w]),
                            stop=(t == last[w]),
                            tile_position=(0, 64 * half),
                        )
                        if t == last[2 * pair + 1]:
                            nc.scalar.activation(
                                out=xnew[:, pair * 64:(pair + 1) * 64],
                                in_=pt[:],
                                func=mybir.ActivationFunctionType.Copy,
                            )
                            del psum_tiles[pair]
                assert not psum_tiles
                nc.sync.dma_start(out=xouts[layer][:], in_=xnew[:])
                if layer < n_layers - 1:
                    nc.sync.dma_start(out=slab_bounce[:], in_=xnew[:])
                    nc.gpsimd.collective_compute(
                        "AllGather", mybir.AluOpType.bypass,
                        replica_groups=rg,
                        ins=[slab_bounce.opt()],
                        outs=[xgs[layer + 1].opt()],
                    )

    nc.compile()
    return nc


def kernel(user_idx, item_idx, rows, cols, values, user_emb, item_emb):
    user_idx = np.asarray(user_idx)
    item_idx = np.asarray(item_idx)
    rows = np.asarray(rows)
    cols = np.asarray(cols)
    values = np.asarray(values, dtype=np.float32)
    user_emb = np.asarray(user_emb, dtype=np.float32)
    item_emb = np.asarray(item_emb, dtype=np.float32)

    x0 = np.zeros((N_PAD, D), dtype=np.float32)
    x0[:NUM_USERS] = user_emb
    x0[NUM_USERS:N_NODES] = item_emb

    plan = _build_plan(rows, cols, values, N_PAD, ck=64)
    nc = _build_nc(plan)

    x0_w = _wrap_slabs(x0.astype(ml_dtypes.bfloat16), N_PAD)
    Gpad, ck = plan["Gpad"], plan["ck"]
    chunks = Gpad // ck
    in_maps = []
    for c in range(NCORES):
        idxc = plan["idx"][c].reshape(chunks, ck, 128).transpose(0, 2, 1)
        in_maps.append({
            "x0slab": np.ascontiguousarray(x0_w[c]),
            "ind": np.ascontiguousarray(
                plan["ind"][c].reshape(128, Gpad * WINDOW)),
            "eidx": np.ascontiguousarray(idxc),
        })

    res = run_bass_kernel_spmd(nc, in_maps, list(range(NCORES)))

    slab = plan["slab"]
    acc = x0.copy()
    for c in range(NCORES):
        s = np.zeros((slab, D), dtype=np.float32)
        for layer in range(N_LAYERS):
            s += _unwrap_slab(
                res.results[c][f"xout{layer}"].astype(np.float32), slab)
        acc[c * slab:(c + 1) * slab] += s
    light = acc[:N_NODES] / (N_LAYERS + 1.0)

    user = light[user_idx]
    item = light[NUM_USERS + item_idx]
    rating = np.sum(user * item, axis=1)
    return (rating.astype(np.float32), user.astype(np.float32),
            item.astype(np.float32))


# revision 5
# speedup vs baseline: 1.4175x; 1.4175x over previous
"""LightGCN-style 3-layer sparse propagation on TRN2 (8 NeuronCores).

Row-sharded SpMM: each core owns a contiguous slab of output rows. Edges
are sorted by destination row, grouped into 64-row windows, and padded
into 128-edge tiles. A host-prebuilt values-folded one-hot indicator
[128 edges x 64 window-rows] (bf16, SBUF-resident) turns the per-window
segment-sum into a TensorE matmul accumulating in PSUM (one PSUM bank
per window pair, col-tiling for odd windows). Edge source rows are
gathered from a bf16 replica of x in DRAM via indirect DMA (one 128-row
tile per instruction). Between layers the new slabs are AllGathered.
Per-layer slabs are returned to the host, which sums layers, scales by
1/4, and does the final batch lookups (O(B*d), negligible).
"""

import math
import sys

import numpy as np

for _p in ("/root/.axon_site", "/root/.axon_site/_ro/trn_rl_repo",
           "/root/.axon_site/_ro/pypackages", "/opt/trn_rl_repo"):
    if _p not in sys.path:
        sys.path.append(_p)

import ml_dtypes

import concourse.bass as bass
import concourse.bacc as bacc
import concourse.tile as tile
from concourse import mybir
from concourse.bass_utils import run_bass_kernel_spmd

D = 64
WINDOW = 64
NCORES = 8
N_LAYERS = 3

NUM_USERS = 100000
NUM_ITEMS = 50000
N_NODES = NUM_USERS + NUM_ITEMS
N_PAD = 150528  # 8 * 18816; 18816 = 294*64 = 147*128

BF16 = mybir.dt.bfloat16
F32 = mybir.dt.float32
I32 = mybir.dt.int32


def _build_plan(rows, cols, values, n_pad, ck=64, src_n_pad=None):
    if src_n_pad is None:
        src_n_pad = n_pad
    slab = n_pad // NCORES
    src_slab = src_n_pad // NCORES
    src_blkn = src_slab // 128
    nwin = slab // WINDOW
    blkn = slab // 128

    nnz = rows.shape[0]
    order = np.argsort(rows, kind="stable")
    r_s = rows[order].astype(np.int64)
    c_s = cols[order].astype(np.int64)
    v_s = values[order].astype(np.float32)

    gw = r_s // WINDOW
    group_sizes = np.bincount(gw, minlength=NCORES * nwin)
    counts = group_sizes.reshape(NCORES, nwin)
    T = np.maximum(1, np.ceil(counts.max(axis=0) / 128.0).astype(np.int64))
    G = int(T.sum())
    Gpad = ((G + ck - 1) // ck) * ck
    T = T.copy()
    T[-1] += Gpad - G
    first = np.concatenate([[0], np.cumsum(T)[:-1]]).astype(np.int64)
    last = (np.cumsum(T) - 1).astype(np.int64)
    win_of_tile = np.repeat(np.arange(nwin), T)

    group_start = np.concatenate([[0], np.cumsum(group_sizes)[:-1]])
    ordinal = np.arange(nnz) - group_start[gw]
    tile_within = ordinal // 128
    p = ordinal % 128
    core = gw // nwin
    local_w = gw % nwin
    tile_global = first[local_w] + tile_within
    local_row = r_s % WINDOW

    cv = c_s // src_slab
    rl = c_s % src_slab
    gidx = ((cv * 128 + rl % 128) * src_blkn + rl // 128).astype(np.int32)

    idx = np.zeros((NCORES, Gpad, 128), dtype=np.int32)
    ind = np.zeros((NCORES, Gpad, 128, WINDOW), dtype=ml_dtypes.bfloat16)
    idx[core, tile_global, p] = gidx
    ind[core, tile_global, p, local_row] = v_s
    ind = np.ascontiguousarray(ind.transpose(0, 2, 1, 3))

    return dict(
        slab=slab, nwin=nwin, blkn=blkn, Gpad=Gpad, ck=ck,
        T=T, first=first, last=last, win_of_tile=win_of_tile,
        idx=idx, ind=ind,
    )


def _wrap_slabs(x, n_pad):
    slab = n_pad // NCORES
    blkn = slab // 128
    return np.ascontiguousarray(
        x.reshape(NCORES, blkn, 128, D).transpose(0, 2, 1, 3)
        .reshape(NCORES, 128, blkn * D)
    )


def _unwrap_slab(xw, slab):
    blkn = slab // 128
    return xw.reshape(128, blkn, D).transpose(1, 0, 2).reshape(slab, D)


def _build_nc(plan, planB, n_layers=N_LAYERS):
    nwin = plan["nwin"]
    blkn = plan["blkn"]
    Gpad = plan["Gpad"]
    ck = plan["ck"]
    first = plan["first"]
    last = plan["last"]
    win_of_tile = plan["win_of_tile"]
    slab_free = blkn * D
    chunks = Gpad // ck

    nc = bacc.Bacc("TRN2", target_bir_lowering=False, debug=False,
                   num_devices=NCORES)

    x0_d = nc.dram_tensor("x0slab", [128, slab_free], BF16,
                          kind="ExternalInput").ap()
    ind_d = nc.dram_tensor("ind", [128, Gpad * WINDOW], BF16,
                           kind="ExternalInput").ap()
    eidx_d = nc.dram_tensor("eidx", [chunks, 128, ck], I32,
                            kind="ExternalInput").ap()
    GpadB, ckB = planB["Gpad"], planB["ck"]
    chunksB = GpadB // ckB
    slab_freeB = planB["blkn"] * D
    indB_d = nc.dram_tensor("ind3", [128, GpadB * WINDOW], BF16,
                            kind="ExternalInput").ap()
    eidxB_d = nc.dram_tensor("eidx3", [chunksB, 128, ckB], I32,
                             kind="ExternalInput").ap()
    xouts = [
        nc.dram_tensor(f"xout{layer}", [128, slab_free], BF16,
                       kind="ExternalOutput").ap()
        for layer in range(n_layers - 1)
    ]
    xoutB = nc.dram_tensor(f"xout{n_layers - 1}", [128, slab_freeB], BF16,
                           kind="ExternalOutput").ap()

    rg = [list(range(NCORES))]

    with tile.TileContext(nc) as tc:
        with tc.tile_pool(name="dram", bufs=1, space="DRAM") as dpool, \
             tc.tile_pool(name="const", bufs=1) as cpool, \
             tc.tile_pool(name="xnew", bufs=1) as xpool, \
             tc.tile_pool(name="gt", bufs=2) as gpool, \
             tc.tile_pool(name="idx", bufs=3) as ipool, \
             tc.tile_pool(name="ps", bufs=8, space="PSUM") as pspool:

            slab_bounce = dpool.tile([128, slab_free], BF16)
            xgs = [
                dpool.tile([128 * NCORES, slab_free], BF16,
                           addr_space="Shared", name=f"xg{layer}")
                for layer in range(n_layers)
            ]
            xvs = [xg.rearrange("a (b c) -> (a b) c", c=D) for xg in xgs]

            ind_sb = cpool.tile([128, Gpad * WINDOW], BF16)
            nc.sync.dma_start(out=ind_sb[:], in_=ind_d[:])
            indB_sb = cpool.tile([128, GpadB * WINDOW], BF16)
            nc.sync.dma_start(out=indB_sb[:], in_=indB_d[:])
            nc.sync.dma_start(out=slab_bounce[:], in_=x0_d[:])
            nc.gpsimd.collective_compute(
                "AllGather", mybir.AluOpType.bypass, replica_groups=rg,
                ins=[slab_bounce.opt()], outs=[xgs[0].opt()],
            )

            for layer in range(n_layers - 1):
                xnew = xpool.tile([128, slab_free], BF16)
                psum_tiles = {}
                for chi in range(chunks):
                    itile = ipool.tile([128, ck], I32)
                    nc.sync.dma_start(out=itile[:], in_=eidx_d[chi])
                    gtile = gpool.tile([128, ck * D], BF16)
                    for k in range(ck):
                        nc.gpsimd.indirect_dma_start(
                            out=gtile[:, k * D:(k + 1) * D],
                            out_offset=None,
                            in_=xvs[layer],
                            in_offset=bass.IndirectOffsetOnAxis(
                                ap=itile[:, k:k + 1], axis=0),
                        )
                    for k in range(ck):
                        t = chi * ck + k
                        w = int(win_of_tile[t])
                        pair = w // 2
                        half = w % 2
                        if pair not in psum_tiles:
                            psum_tiles[pair] = pspool.tile(
                                [128, 64], F32,
                                name=f"ps_l{layer}_p{pair}", tag="ps")
                        pt = psum_tiles[pair]
                        nc.tensor.matmul(
                            out=pt[64 * half:64 * half + 64, :],
                            lhsT=ind_sb[:, t * WINDOW:(t + 1) * WINDOW],
                            rhs=gtile[:, k * D:(k + 1) * D],
                            start=(t == first[w]),
                            stop=(t == last[w]),
                            tile_position=(0, 64 * half),
                        )
                        if t == last[2 * pair + 1]:
                            nc.scalar.activation(
                                out=xnew[:, pair * 64:(pair + 1) * 64],
                                in_=pt[:],
                                func=mybir.ActivationFunctionType.Copy,
                            )
                            del psum_tiles[pair]
                assert not psum_tiles
                nc.sync.dma_start(out=xouts[layer][:], in_=xnew[:])
                nc.sync.dma_start(out=slab_bounce[:], in_=xnew[:])
                nc.gpsimd.collective_compute(
                    "AllGather", mybir.AluOpType.bypass,
                    replica_groups=rg,
                    ins=[slab_bounce.opt()],
                    outs=[xgs[layer + 1].opt()],
                )

            # final layer: compact output (only batch-needed rows)
            firstB, lastB = planB["first"], planB["last"]
            wotB = planB["win_of_tile"]
            xnewB = xpool.tile([128, slab_freeB], BF16, name="xnewB")
            psum_tiles = {}
            for chi in range(chunksB):
                itile = ipool.tile([128, ckB], I32, name="itileB")
                nc.sync.dma_start(out=itile[:], in_=eidxB_d[chi])
                gtile = gpool.tile([128, ckB * D], BF16, name="gtileB")
                for k in range(ckB):
                    nc.gpsimd.indirect_dma_start(
                        out=gtile[:, k * D:(k + 1) * D],
                        out_offset=None,
                        in_=xvs[n_layers - 1],
                        in_offset=bass.IndirectOffsetOnAxis(
                            ap=itile[:, k:k + 1], axis=0),
                    )
                for k in range(ckB):
                    t = chi * ckB + k
                    w = int(wotB[t])
                    pair = w // 2
                    half = w % 2
                    if pair not in psum_tiles:
                        psum_tiles[pair] = pspool.tile(
                            [128, 64], F32, name=f"ps_lB_p{pair}", tag="ps")
                    pt = psum_tiles[pair]
                    nc.tensor.matmul(
                        out=pt[64 * half:64 * half + 64, :],
                        lhsT=indB_sb[:, t * WINDOW:(t + 1) * WINDOW],
                        rhs=gtile[:, k * D:(k + 1) * D],
                        start=(t == firstB[w]),
                        stop=(t == lastB[w]),
                        tile_position=(0, 64 * half),
                    )
                    if t == lastB[2 * pair + 1]:
                        nc.scalar.activation(
                            out=xnewB[:, pair * 64:(pair + 1) * 64],
                            in_=pt[:],
                            func=mybir.ActivationFunctionType.Copy,
                        )
                        del psum_tiles[pair]
            assert not psum_tiles
            nc.sync.dma_start(out=xoutB[:], in_=xnewB[:])

    nc.compile()
    return nc


def kernel(user_idx, item_idx, rows, cols, values, user_emb, item_emb):
    user_idx = np.asarray(user_idx)
    item_idx = np.asarray(item_idx)
    rows = np.asarray(rows)
    cols = np.asarray(cols)
    values = np.asarray(values, dtype=np.float32)
    user_emb = np.asarray(user_emb, dtype=np.float32)
    item_emb = np.asarray(item_emb, dtype=np.float32)

    x0 = np.zeros((N_PAD, D), dtype=np.float32)
    x0[:NUM_USERS] = user_emb
    x0[NUM_USERS:N_NODES] = item_emb

    plan = _build_plan(rows, cols, values, N_PAD, ck=64)

    # last layer computes only the batch-needed rows, compact-renumbered
    N_PAD_B = 8192
    needed = np.unique(np.concatenate([
        np.asarray(user_idx, dtype=np.int64),
        NUM_USERS + np.asarray(item_idx, dtype=np.int64)]))
    Nn = len(needed)
    assert Nn <= N_PAD_B
    ss = np.searchsorted(needed, rows)
    ssc = np.minimum(ss, Nn - 1)
    mask = needed[ssc] == rows
    rowsB = ssc[mask].astype(np.int64)
    planB = _build_plan(rowsB, cols[mask], values[mask], N_PAD_B,
                        ck=32, src_n_pad=N_PAD)
    nc = _build_nc(plan, planB)

    x0_w = _wrap_slabs(x0.astype(ml_dtypes.bfloat16), N_PAD)
    Gpad, ck = plan["Gpad"], plan["ck"]
    chunks = Gpad // ck
    GpadB, ckB = planB["Gpad"], planB["ck"]
    chunksB = GpadB // ckB
    in_maps = []
    for c in range(NCORES):
        idxc = plan["idx"][c].reshape(chunks, ck, 128).transpose(0, 2, 1)
        idxcB = planB["idx"][c].reshape(chunksB, ckB, 128).transpose(0, 2, 1)
        in_maps.append({
            "x0slab": np.ascontiguousarray(x0_w[c]),
            "ind": np.ascontiguousarray(
                plan["ind"][c].reshape(128, Gpad * WINDOW)),
            "eidx": np.ascontiguousarray(idxc),
            "ind3": np.ascontiguousarray(
                planB["ind"][c].reshape(128, GpadB * WINDOW)),
            "eidx3": np.ascontiguousarray(idxcB),
        })

    res = run_bass_kernel_spmd(nc, in_maps, list(range(NCORES)))

    slab = plan["slab"]
    acc = x0.copy()
    for c in range(NCORES):
        s = np.zeros((slab, D), dtype=np.float32)
        for layer in range(N_LAYERS - 1):
            s += _unwrap_slab(
                res.results[c][f"xout{layer}"].astype(np.float32), slab)
        acc[c * slab:(c + 1) * slab] += s
    # compact last-layer slabs -> values at `needed` rows
    slabB = planB["slab"]
    x3c = np.concatenate([
        _unwrap_slab(res.results[c][f"xout{N_LAYERS - 1}"].astype(np.float32),
                     slabB)
        for c in range(NCORES)], axis=0)
    vals = (acc[needed] + x3c[:Nn]) / (N_LAYERS + 1.0)

    user = vals[np.searchsorted(needed, np.asarray(user_idx, dtype=np.int64))]
    item = vals[np.searchsorted(needed,
                                NUM_USERS + np.asarray(item_idx, dtype=np.int64))]
    rating = np.sum(user * item, axis=1)
    return (rating.astype(np.float32), user.astype(np.float32),
            item.astype(np.float32))


# revision 8
# speedup vs baseline: 1.9553x; 1.3794x over previous
"""LightGCN-style 3-layer sparse propagation on TRN2 (8 NeuronCores).

Row-sharded SpMM: each core owns a contiguous slab of output rows. Edges
are sorted by destination row, grouped into 64-row windows, and padded
into 128-edge tiles. A host-prebuilt values-folded one-hot indicator
[128 edges x 64 window-rows] (bf16, SBUF-resident) turns the per-window
segment-sum into a TensorE matmul accumulating in PSUM (one PSUM bank
per window pair, col-tiling for odd windows). Edge source rows are
gathered from a bf16 replica of x in DRAM via indirect DMA (one 128-row
tile per instruction). Between layers the new slabs are AllGathered.
Per-layer slabs are returned to the host, which sums layers, scales by
1/4, and does the final batch lookups (O(B*d), negligible).
"""

import math
import sys

import numpy as np

for _p in ("/root/.axon_site", "/root/.axon_site/_ro/trn_rl_repo",
           "/root/.axon_site/_ro/pypackages", "/opt/trn_rl_repo"):
    if _p not in sys.path:
        sys.path.append(_p)

import ml_dtypes

import concourse.bass as bass
import concourse.bacc as bacc
import concourse.tile as tile
from concourse import mybir
from concourse.bass_utils import run_bass_kernel_spmd

D = 64
WINDOW = 64
NCORES = 8
N_LAYERS = 3

NUM_USERS = 100000
NUM_ITEMS = 50000
N_NODES = NUM_USERS + NUM_ITEMS
N_PAD = 150528  # 8 * 18816; 18816 = 294*64 = 147*128

BF16 = mybir.dt.bfloat16
F32 = mybir.dt.float32
I32 = mybir.dt.int32


def _build_plan(rows, cols, values, n_pad, ck=64, src_n_pad=None):
    if src_n_pad is None:
        src_n_pad = n_pad
    slab = n_pad // NCORES
    src_slab = src_n_pad // NCORES
    src_blkn = src_slab // 128
    nwin = slab // WINDOW
    blkn = slab // 128

    nnz = rows.shape[0]
    order = np.argsort(rows, kind="stable")
    r_s = rows[order].astype(np.int64)
    c_s = cols[order].astype(np.int64)
    v_s = values[order].astype(np.float32)

    gw = r_s // WINDOW
    group_sizes = np.bincount(gw, minlength=NCORES * nwin)
    counts = group_sizes.reshape(NCORES, nwin)
    T = np.maximum(1, np.ceil(counts.max(axis=0) / 128.0).astype(np.int64))
    G = int(T.sum())
    Gpad = ((G + ck - 1) // ck) * ck
    T = T.copy()
    T[-1] += Gpad - G
    first = np.concatenate([[0], np.cumsum(T)[:-1]]).astype(np.int64)
    last = (np.cumsum(T) - 1).astype(np.int64)
    win_of_tile = np.repeat(np.arange(nwin), T)

    group_start = np.concatenate([[0], np.cumsum(group_sizes)[:-1]])
    ordinal = np.arange(nnz) - group_start[gw]
    tile_within = ordinal // 128
    p = ordinal % 128
    core = gw // nwin
    local_w = gw % nwin
    tile_global = first[local_w] + tile_within
    local_row = r_s % WINDOW

    cv = c_s // src_slab
    rl = c_s % src_slab
    gidx = ((cv * 128 + rl % 128) * src_blkn + rl // 128).astype(np.int32)

    idx = np.zeros((NCORES, Gpad, 128), dtype=np.int32)
    ind = np.zeros((NCORES, Gpad, 128, WINDOW), dtype=ml_dtypes.bfloat16)
    idx[core, tile_global, p] = gidx
    ind[core, tile_global, p, local_row] = v_s
    ind = np.ascontiguousarray(ind.transpose(0, 2, 1, 3))

    return dict(
        slab=slab, nwin=nwin, blkn=blkn, Gpad=Gpad, ck=ck,
        T=T, first=first, last=last, win_of_tile=win_of_tile,
        idx=idx, ind=ind,
    )


def _wrap_slabs(x, n_pad):
    slab = n_pad // NCORES
    blkn = slab // 128
    return np.ascontiguousarray(
        x.reshape(NCORES, blkn, 128, D).transpose(0, 2, 1, 3)
        .reshape(NCORES, 128, blkn * D)
    )


def _unwrap_slab(xw, slab):
    blkn = slab // 128
    return xw.reshape(128, blkn, D).transpose(1, 0, 2).reshape(slab, D)


def _build_nc(plans, n_layers=N_LAYERS):
    """plans = [planA (full l0), plan2 (compact l1), planB (compact l2)]."""
    assert len(plans) == n_layers
    nc = bacc.Bacc("TRN2", target_bir_lowering=False, debug=False,
                   num_devices=NCORES)

    slab_frees = [p["blkn"] * D for p in plans]
    x0_d = nc.dram_tensor("x0slab", [128, slab_frees[0]], BF16,
                          kind="ExternalInput").ap()
    ind_ds, eidx_ds, xout_ds = [], [], []
    for li, p in enumerate(plans):
        ch = p["Gpad"] // p["ck"]
        ind_ds.append(nc.dram_tensor(f"ind{li}", [128, p["Gpad"] * WINDOW],
                                     BF16, kind="ExternalInput").ap())
        eidx_ds.append(nc.dram_tensor(f"eidx{li}", [ch, 128, p["ck"]], I32,
                                      kind="ExternalInput").ap())
        xout_ds.append(nc.dram_tensor(f"xout{li}", [128, slab_frees[li]],
                                      BF16, kind="ExternalOutput").ap())

    rg = [list(range(NCORES))]

    with tile.TileContext(nc) as tc:
        with tc.tile_pool(name="dram", bufs=1, space="DRAM") as dpool, \
             tc.tile_pool(name="const", bufs=1) as cpool, \
             tc.tile_pool(name="xnew", bufs=1) as xpool, \
             tc.tile_pool(name="gt", bufs=2) as gpool, \
             tc.tile_pool(name="idx", bufs=3) as ipool, \
             tc.tile_pool(name="ps", bufs=8, space="PSUM") as pspool:

            # xg[li] holds the AllGathered INPUT of layer li, in the layout
            # of the layer's source plan (full for l0/l1, compact2 for l2)
            xg_frees = [slab_frees[0], slab_frees[0], slab_frees[1]]
            bounces = [
                dpool.tile([128, xg_frees[li]], BF16, name=f"bounce{li}")
                for li in range(n_layers)
            ]
            xgs = [
                dpool.tile([128 * NCORES, xg_frees[li]], BF16,
                           addr_space="Shared", name=f"xg{li}")
                for li in range(n_layers)
            ]
            xvs = [xg.rearrange("a (b c) -> (a b) c", c=D) for xg in xgs]

            nc.sync.dma_start(out=bounces[0][:], in_=x0_d[:])
            nc.gpsimd.collective_compute(
                "AllGather", mybir.AluOpType.bypass, replica_groups=rg,
                ins=[bounces[0].opt()], outs=[xgs[0].opt()],
            )

            for li, p in enumerate(plans):
                ck = p["ck"]
                chunks = p["Gpad"] // ck
                first, last, wot = p["first"], p["last"], p["win_of_tile"]
                xnew = xpool.tile([128, slab_frees[li]], BF16,
                                  name=f"xnew{li}", tag="xn",
                                  padded_shape=[128, slab_frees[0]])
                psum_tiles = {}
                for chi in range(chunks):
                    itile = ipool.tile([128, ck], I32, name=f"it{li}",
                                       tag="it")
                    nc.sync.dma_start(out=itile[:], in_=eidx_ds[li][chi])
                    ind_t = cpool.tile([128, ck * WINDOW], BF16,
                                       name=f"ind{li}", tag="inds", bufs=3)
                    nc.sync.dma_start(
                        out=ind_t[:],
                        in_=ind_ds[li][:, chi * ck * WINDOW:
                                       (chi + 1) * ck * WINDOW])
                    gtile = gpool.tile([128, ck * D], BF16, name=f"gt{li}",
                                       tag="gt")
                    for k in range(ck):
                        nc.gpsimd.indirect_dma_start(
                            out=gtile[:, k * D:(k + 1) * D],
                            out_offset=None,
                            in_=xvs[li],
                            in_offset=bass.IndirectOffsetOnAxis(
                                ap=itile[:, k:k + 1], axis=0),
                        )
                    for k in range(ck):
                        t = chi * ck + k
                        w = int(wot[t])
                        pair = w // 2
                        half = w % 2
                        if pair not in psum_tiles:
                            psum_tiles[pair] = pspool.tile(
                                [128, 64], F32,
                                name=f"ps_l{li}_p{pair}", tag="ps")
                        pt = psum_tiles[pair]
                        nc.tensor.matmul(
                            out=pt[64 * half:64 * half + 64, :],
                            lhsT=ind_t[:, k * WINDOW:(k + 1) * WINDOW],
                            rhs=gtile[:, k * D:(k + 1) * D],
                            start=(t == first[w]),
                            stop=(t == last[w]),
                            tile_position=(0, 64 * half),
                        )
                        if t == last[2 * pair + 1]:
                            nc.scalar.activation(
                                out=xnew[:, pair * 64:(pair + 1) * 64],
                                in_=pt[:],
                                func=mybir.ActivationFunctionType.Copy,
                            )
                            del psum_tiles[pair]
                assert not psum_tiles
                nc.sync.dma_start(out=xout_ds[li][:], in_=xnew[:])
                if li < n_layers - 1:
                    nc.sync.dma_start(out=bounces[li + 1][:], in_=xnew[:])
                    nc.gpsimd.collective_compute(
                        "AllGather", mybir.AluOpType.bypass,
                        replica_groups=rg,
                        ins=[bounces[li + 1].opt()],
                        outs=[xgs[li + 1].opt()],
                    )

    nc.compile()
    return nc


def kernel(user_idx, item_idx, rows, cols, values, user_emb, item_emb):
    user_idx = np.asarray(user_idx)
    item_idx = np.asarray(item_idx)
    rows = np.asarray(rows)
    cols = np.asarray(cols)
    values = np.asarray(values, dtype=np.float32)
    user_emb = np.asarray(user_emb, dtype=np.float32)
    item_emb = np.asarray(item_emb, dtype=np.float32)

    x0 = np.zeros((N_PAD, D), dtype=np.float32)
    x0[:NUM_USERS] = user_emb
    x0[NUM_USERS:N_NODES] = item_emb

    # Layer 0: full SpMM. Layer 2 output: only batch-needed rows (compact).
    # Layer 1 output: only rows consumed downstream = union of layer-2 edge
    # source cols and the needed rows (compact2).
    N_PAD_B = 8192
    needed = np.unique(np.concatenate([
        np.asarray(user_idx, dtype=np.int64),
        NUM_USERS + np.asarray(item_idx, dtype=np.int64)]))
    Nn = len(needed)
    assert Nn <= N_PAD_B

    def member(sorted_arr, x):
        ss = np.searchsorted(sorted_arr, x)
        ssc = np.minimum(ss, len(sorted_arr) - 1)
        return ssc, sorted_arr[ssc] == x

    pos3, mask3 = member(needed, rows)
    S2 = np.unique(np.concatenate([needed, cols[mask3].astype(np.int64)]))
    N2 = len(S2)
    n_pad2 = ((N2 + 1023) // 1024) * 1024

    planA = _build_plan(rows, cols, values, N_PAD, ck=64)
    pos2, mask2 = member(S2, rows)
    plan2 = _build_plan(pos2[mask2].astype(np.int64), cols[mask2],
                        values[mask2], n_pad2, ck=64, src_n_pad=N_PAD)
    c3, c3ok = member(S2, cols[mask3])
    assert c3ok.all()
    planB = _build_plan(pos3[mask3].astype(np.int64), c3.astype(np.int64),
                        values[mask3], N_PAD_B, ck=32, src_n_pad=n_pad2)
    plans = [planA, plan2, planB]
    nc = _build_nc(plans)

    x0_w = _wrap_slabs(x0.astype(ml_dtypes.bfloat16), N_PAD)
    in_maps = []
    for c in range(NCORES):
        m = {"x0slab": np.ascontiguousarray(x0_w[c])}
        for li, p in enumerate(plans):
            Gp, ckp = p["Gpad"], p["ck"]
            ch = Gp // ckp
            idxc = p["idx"][c].reshape(ch, ckp, 128).transpose(0, 2, 1)
            m[f"ind{li}"] = np.ascontiguousarray(
                p["ind"][c].reshape(128, Gp * WINDOW))
            m[f"eidx{li}"] = np.ascontiguousarray(idxc)
        in_maps.append(m)

    res = run_bass_kernel_spmd(nc, in_maps, list(range(NCORES)))

    def collect(li):
        p = plans[li]
        return np.concatenate([
            _unwrap_slab(res.results[c][f"xout{li}"].astype(np.float32),
                         p["slab"])
            for c in range(NCORES)], axis=0)

    x1f = collect(0)            # full [N_PAD, D]
    x2c = collect(1)            # compact2 [n_pad2, D]
    x3c = collect(2)            # compact [8192, D]
    vals = (x0[needed] + x1f[needed]
            + x2c[np.searchsorted(S2, needed)]
            + x3c[:Nn]) / (N_LAYERS + 1.0)

    user = vals[np.searchsorted(needed, np.asarray(user_idx, dtype=np.int64))]
    item = vals[np.searchsorted(needed,
                                NUM_USERS + np.asarray(item_idx,
                                                       dtype=np.int64))]
    rating = np.sum(user * item, axis=1)
    return (rating.astype(np.float32), user.astype(np.float32),
            item.astype(np.float32))
